# revision 61
# baseline (speedup 1.0000x reference)
"""Trainium2 Bass kernel for the SCON linear-SDE particle scan (v2).

Reference computation: x_{t+1} = (I + DT*W_{t+1}) x_t + DT*b_{t+1} + ds*eps_t
over 10000 steps for B=512 particles with a 3-dim state, observed every 50
steps through a [4,3] projection -> loc_y [512, 201, 4].

The transition matrices depend only on theta (14 scalars), so the whole scan
is a linear map of (x0, eps).  On the host (float64) we precompute propagator
weights; the device reduces the noise tensor in two levels of bf16 PE matmuls
(rel tolerance is 2e-2; bf16 keeps the L2 rel err at ~3e-3):

  level AB: chunk weights G2[c] = S50[w(c),g(c)] @ S10[c] * ds fold the
            10-step chunk propagator AND the within-window chunk propagator,
            so 260 matmuls over the noise produce u50 (per-window noise
            state) directly in PSUM accumulation groups.
  level C:  triangular window->observation propagation + obs projection,
            interleaved per window-tile as u50 tiles complete.

The deterministic/x0 affine part runs as two fp32 K=4 matmuls (x0aug.T @
RXaug) that open the level-C PSUM accumulation groups.  Output halves are
copied out and stored in stages as their contributing window-tiles finish.

B is sharded 64 particles per core across 8 cores (pure data parallel).
Per-core device traffic: ~4.1 MB noise (bf16) + ~1.9 MB weights.  All big
transfers stream on ONE sync-HWDGE FIFO in compute order (g2/eps slices
with per-tile level-C weight chunks inlined before their consumers); the
scalar ring carries only tiny x0 operands and the staged output stores, so
recycling of the 8 DMA completion lanes can never stall a stream kickoff.
"""

import numpy as np
import ml_dtypes

# ---------------------------------------------------------------- constants
T_TOT = 1000.0
DT = 0.1
N = 10001
TEMP_REF = 283.0
TEMP_RISE = 5.0
GAS_R = 0.008314
NSTEP = N - 1            # 10000
B = 512
NCORE = 8
BC = B // NCORE          # 64 particles per core

L1 = 10                  # chunk length (steps)
NC1 = NSTEP // L1        # 1000 chunks
CPW = 5                  # chunks per window
NW = NC1 // CPW          # 200 windows
NOBS = NW + 1            # 201 observations
OBS_EVERY = 50
SUPER = 4                # chunks per matmul (4 x 32 eps rows)
NSUP = NC1 // SUPER      # 250 supergroups
NTILE_B = 5              # u50 window-tiles (40 windows each)
WPB = 10                 # windows per 32-partition block

NOUT = 4 * NOBS          # 804
NH = NOUT // 2           # 402

# stream DMA slices (sup counts), all on the sync HWDGE ring in compute
# order.  Big first slices keep the SDMA engines from starving between
# ~0.65us descriptor-generation kickoffs (the PE start time is slack);
# a small last slice keeps the post-stream tail short.
SLICE_SUPS = [6, 10] + [25] * 8 + [20, 14]
SLICE_OFF = np.cumsum([0] + SLICE_SUPS).tolist()
NSLICE = len(SLICE_SUPS)

_program_cache = None
_last_results = None     # BassKernelResults of the most recent run (for test.py)


# ----------------------------------------------------- static piece metadata
def _sup_pieces():
    """Merged A+B matmul pieces.

    Sup s covers chunks 4s..4s+3 -> windows (4s)//5..(4s+3)//5, all within
    window-tile wt (psum cols 64*wt).  Window w maps to psum partition
    32*((w-40wt)//10) + 3*((w-40wt)%10) + i.  A sup whose windows straddle a
    32-partition block is split into two pieces.  Matmul out partitions must
    start 32-aligned, so each piece's lhsT spans block-row 0..cols (leading
    zero weight columns).  All pieces run start=False against a zeroed PSUM
    bank (accumulate-onto-0 and overwrite are equivalent there).
    """
    pieces = []
    for s in range(NSUP):
        ws = [(4 * s + g) // 5 for g in range(SUPER)]
        wt = ws[0] // 40
        by_m = {}
        for g, w in zip(range(SUPER), ws):
            m = (w - 40 * wt) // WPB
            by_m.setdefault(m, []).append(g)
        for m in sorted(by_m):
            gs = by_m[m]
            whi = ws[gs[-1]]
            rend = 3 * ((whi - 40 * wt) % WPB) + 3
            pieces.append(dict(s=s, wt=wt, m=m, gs=gs, rend=rend))
    for p in pieces:
        # pb is memset to zero, so every matmul can run start=False: rows
        # first touched by a matmul either overwrite or accumulate onto 0.
        p['start'] = False
        p['cols'] = p['rend']
        p['slice'] = int(np.searchsorted(SLICE_OFF, p['s'], side='right')) - 1
    last = {}
    for i, p in enumerate(pieces):
        last[(p['wt'], p['m'])] = i
    for i, p in enumerate(pieces):
        p['stop'] = last[(p['wt'], p['m'])] == i
    cur = {}
    for p in pieces:
        k = p['slice']
        p['off'] = cur.get(k, 0)
        cur[k] = p['off'] + p['cols']
    slice_cols = [cur.get(k, 0) for k in range(len(SLICE_SUPS))]
    return pieces, slice_cols


PIECES, SLICE_COLS = _sup_pieces()
CTOT = sum(SLICE_COLS)


def _rsb_blocks():
    """Nonzero column ranges of each level-C (wt, half) block.

    Window-tile wt covers windows [40wt, 40wt+40); its rows only affect
    observations n >= 40wt+1, i.e. global cols >= 4*(40wt+1).  Returns
    (wt, h, rel0, keep, packed_col_offset)."""
    blocks = []
    off = 0
    for wt in range(NTILE_B):          # wt-major: per-tile contiguous cols
        for h in range(2):
            rel0 = max(0, 4 * (40 * wt + 1) - NH * h)
            if rel0 >= NH:
                continue
            keep = NH - rel0
            blocks.append((wt, h, rel0, keep, off))
            off += keep
    return blocks


RSB_BLOCKS = _rsb_blocks()
NRSB = sum(b[3] for b in RSB_BLOCKS)
RSB_WT_RANGE = {}
for wt, h, rel0, keep, off in RSB_BLOCKS:
    o0, o1 = RSB_WT_RANGE.get(wt, (off, off))
    RSB_WT_RANGE[wt] = (min(o0, off), max(o1, off + keep))
# issue plan per window-tile: (h, rel0, keep, off, start, stop)
CBLOCKS_BY_WT = {wt: [] for wt in range(NTILE_B)}
_last_wt_h = {}
for wt, h, rel0, keep, off in RSB_BLOCKS:
    _last_wt_h[h] = max(_last_wt_h.get(h, 0), wt)
for wt, h, rel0, keep, off in RSB_BLOCKS:
    # start=False always: the fp32 x0-part matmul opens each pc group
    CBLOCKS_BY_WT[wt].append((h, rel0, keep, off, False, wt == _last_wt_h[h]))
for wt in CBLOCKS_BY_WT:
    CBLOCKS_BY_WT[wt].sort()

# output staging: after C(wt,h), pc[h] cols [0, rel0(next tile)) are final.
# CSTAGES[(wt, h)] = (a, b): add det+pc on [a, b) and DMA out cols
# [NH*h + a, NH*h + b) right then, so the kernel tail only ships the last
# stage (160 cols) instead of a whole half.
CSTAGES = {}
for h in range(2):
    tiles = sorted([(wt, rel0) for wt, hh, rel0, _, _ in RSB_BLOCKS
                    if hh == h])
    cur = 0
    for i, (wt, rel0) in enumerate(tiles):
        nxt = tiles[i + 1][1] if i + 1 < len(tiles) else NH
        if nxt > cur:
            CSTAGES[(wt, h)] = (cur, nxt)
            cur = nxt


# ------------------------------------------------------------- host math
def _forcings():
    times = np.linspace(0.0, T_TOT, N)
    temp = (TEMP_REF + TEMP_RISE * times / (80 * 24 * 365)
            + 10 * np.sin(2 * np.pi / 24 * times)
            + 10 * np.sin(2 * np.pi / (24 * 365) * times))
    I_S = 0.001 + 0.0005 * np.sin(2 * np.pi / (24 * 365) * times)
    I_D = 0.0001 + 5e-05 * np.sin(2 * np.pi / (24 * 365) * times)
    return temp, I_S, I_D


def _precompute(theta):
    """float64 propagator weights, packed into the device operand layouts."""
    theta = np.asarray(theta, np.float64)
    (kSr, kDr, kMr, EaS, EaD, EaM, aSD, aDS, aM, aMSC, uM, cS, cD, cM) = theta
    temp, I_S, I_D = _forcings()
    arr = lambda p, Ea: p * np.exp(-Ea / GAS_R * (1.0 / temp - 1.0 / TEMP_REF))
    k_S, k_D, k_M = arr(kSr, EaS), arr(kDr, EaD), arr(kMr, EaM)

    zeros = np.zeros(N)
    A0 = np.stack([-k_S, aDS * k_D, aM * aMSC * k_M])
    A1 = np.stack([aSD * k_S, -(uM + k_D), aM * (1 - aMSC) * k_M])
    A2 = np.stack([zeros, np.full(N, uM), -k_M])
    W = np.stack([A0, A1, A2]).transpose(2, 0, 1)          # [N,3,3]
    bias = np.stack([I_S, I_D, zeros], axis=1)             # [N,3]

    beta = np.clip(np.array([cS, cD, cM]), 1e-6, None)
    ds = np.sqrt(beta * DT)

    M = np.eye(3)[None] + DT * W[1:]                       # [10000,3,3]
    c = DT * bias[1:]                                      # [10000,3]

    # within-chunk suffix products S10[c,tau] = M_end ... M_{tau+1}
    Mc = M.reshape(NC1, L1, 3, 3)
    S10 = np.empty((NC1, L1, 3, 3))
    acc = np.broadcast_to(np.eye(3), (NC1, 3, 3)).copy()
    S10[:, L1 - 1] = acc
    for tau in range(L1 - 2, -1, -1):
        acc = acc @ Mc[:, tau + 1]
        S10[:, tau] = acc
    A10 = S10[:, 0] @ Mc[:, 0]
    b10 = np.einsum('ctij,ctj->ci', S10, c.reshape(NC1, L1, 3))

    # within-window suffix products over chunks
    A10w = A10.reshape(NW, CPW, 3, 3)
    S50 = np.empty((NW, CPW, 3, 3))
    acc = np.broadcast_to(np.eye(3), (NW, 3, 3)).copy()
    S50[:, CPW - 1] = acc
    for g in range(CPW - 2, -1, -1):
        acc = acc @ A10w[:, g + 1]
        S50[:, g] = acc
    A50 = S50[:, 0] @ A10w[:, 0]
    b50 = np.einsum('wgij,wgj->wi', S50, b10.reshape(NW, CPW, 3))

    # deterministic trajectory at obs points (exact, float64)
    detx = np.zeros((NOBS, 3))
    xd = np.zeros(3)
    for w in range(NW):
        xd = A50[w] @ xd + b50[w]
        detx[w + 1] = xd

    # merged chunk->u50 weights: G2[c] = S50[w(c),g(c)] @ S10[c] * ds_j
    G2 = np.einsum('cij,ctjk->ctik', S50.reshape(NC1, 3, 3), S10) * ds
    G2mat = G2.transpose(0, 1, 3, 2).reshape(NC1, 30, 3)   # row 3tau+j, col i

    # observation weights
    sub = np.arange(NOBS) * OBS_EVERY
    C1 = np.stack([(1 - aSD) * k_S[sub], (1 - aDS) * k_D[sub],
                   (1 - aM) * k_M[sub]], axis=1)
    Wobs = np.concatenate([np.broadcast_to(np.eye(3), (NOBS, 3, 3)),
                           C1[:, None, :]], axis=1)        # [NOBS,4,3]

    # Rmat[(w,j),(n,o)] = (Wobs[n] @ A50[n-1] ... A50[w+1]).T  for w < n
    Rmat = np.zeros((3 * NW, NOUT))
    base = np.einsum('noi,ni->no', Wobs, detx).reshape(-1)
    acc = Wobs.copy()
    for w in range(NW - 1, -1, -1):
        Rmat[3 * w:3 * w + 3, 4 * (w + 1):] = \
            acc[w + 1:].transpose(2, 0, 1).reshape(3, -1)
        acc[w + 1:] = acc[w + 1:] @ A50[w]
    RX = acc.transpose(2, 0, 1).reshape(3, -1)             # [3, NOUT]
    RXaug = np.concatenate([RX, base[None]], axis=0)       # [4, NOUT] float64

    # ---------------- pack into device layouts ----------------
    bf = ml_dtypes.bfloat16
    g2img = np.zeros((128, CTOT), np.float32)   # chunk g at rows 32g..32g+30
    soff_cols = np.cumsum([0] + SLICE_COLS)
    G2f = np.asarray(G2mat, np.float32)
    for p in PIECES:
        off = soff_cols[p['slice']] + p['off']
        for g in p['gs']:
            ci = 4 * p['s'] + g
            col0 = off + 3 * ((ci // 5 - 40 * p['wt']) % WPB)
            g2img[32 * g:32 * g + 30, col0:col0 + 3] = G2f[ci]

    # u50 row map: window w, comp j -> row 32*((w%40)//10) + 3*(w%10) + j,
    #              col 64*(w//40) + b
    rsb = np.zeros((128, NRSB), np.float32)
    for wt, h, rel0, keep, off in RSB_BLOCKS:
        for rho in range(128):
            q = rho % 32
            if q >= 30:
                continue
            w = WPB * (4 * wt + rho // 32) + q // 3
            j = q % 3
            rsb[rho, off:off + keep] = \
                Rmat[3 * w + j, NH * h + rel0:NH * h + rel0 + keep]

    return dict(g2=g2img.astype(bf), rsb=rsb.astype(bf), RXaug=RXaug)


def _pack_eps(noise_core):
    """[64,10000,3] f32 -> [128, 250*64] bf16: row 32g + (3tau+j),
    col 64s + b = eps[b, t, j] for t = 40s + 10g + tau; rows 32g+30/31 pad."""
    a = noise_core.reshape(BC, NSTEP * 3).T          # [30000, 64] view
    a = np.ascontiguousarray(a).reshape(NSUP, SUPER, 30, BC)
    out = np.zeros((SUPER, 32, NSUP, BC), ml_dtypes.bfloat16)
    out[:, :30] = a.transpose(1, 2, 0, 3).astype(ml_dtypes.bfloat16)
    return out.reshape(128, NSUP * BC)


# ------------------------------------------------------------ bass program
def _build_program(**bass_kwargs):
    import concourse.bass as bass
    import concourse.tile as tile
    from concourse import bacc, mybir

    f32 = mybir.dt.float32
    bf16 = mybir.dt.bfloat16
    nc = bacc.Bacc(None, target_bir_lowering=False, **bass_kwargs)

    # per-slice stream segment: [g2_k cols | eps_k cols], one DMA each.
    # 128 rows with chunk g at rows 32g..32g+30 (rows 32g+30/31 zero pad):
    # 120-partition DMAs measure ~230 GB/s vs ~400 GB/s for 128-partition,
    # so shipping the pad rows is the faster option.
    seg_cols = [SLICE_COLS[k] + BC * SLICE_SUPS[k]
                for k in range(len(SLICE_SUPS))]
    seg_off = np.cumsum([0] + seg_cols)

    stream_d = nc.dram_tensor("stream", [128, int(seg_off[-1])], bf16,
                              kind="ExternalInput")
    rsb_d = nc.dram_tensor("rsb", [128, NRSB], bf16, kind="ExternalInput")
    x0_d = nc.dram_tensor("x0aug", [4, BC], f32, kind="ExternalInput")
    rx_d = nc.dram_tensor("rxaug", [4, NOUT], f32, kind="ExternalInput")
    out_d = nc.dram_tensor("out", [BC, NOUT], f32, kind="ExternalOutput")

    with tile.TileContext(nc) as tc:
        with (
            tc.tile_pool(name="consts", bufs=1) as consts,
            tc.tile_pool(name="epsp", bufs=1) as epsp,
            tc.tile_pool(name="psB", bufs=1, space="PSUM") as psB,
            tc.tile_pool(name="psC", bufs=2, space="PSUM") as psC,
        ):
            rsb = consts.tile([128, NRSB], bf16)
            x0t = consts.tile([4, BC], f32)
            rxt = consts.tile([4, NOUT], f32)
            u50sb = consts.tile([128, NTILE_B * BC], bf16)
            outsb = consts.tile([BC, NOUT], f32)
            seg_t = [epsp.tile([128, sc], bf16, tag=f"seg{k}",
                               name=f"seg{k}")
                     for k, sc in enumerate(seg_cols)]

            # scalar (qAct) HWDGE ring: only the tiny x0 operands (plus the
            # staged output stores later).  Keeping big transfers off this
            # ring stops sync-ring kickoffs from serializing behind them
            # when the 8 DMA completion lanes are recycled.
            nc.scalar.dma_start(out=x0t, in_=x0_d[:])
            nc.scalar.dma_start(out=rxt, in_=rx_d[:])
            # sync (qSP) HWDGE ring: one DMA per slice (weights + noise) in
            # compute order; level-C weight chunks for tile wt are streamed
            # inline just before the slices that complete the tile, keeping
            # all big transfers on one FIFO (in-order completions mean the
            # 8 recycled completion lanes can never stall a later kickoff)
            for k in range(NSLICE):
                nc.sync.dma_start(
                    out=seg_t[k],
                    in_=stream_d[:, int(seg_off[k]):int(seg_off[k + 1])])
                # tile wt completes in slice 2wt+3; its level-C weight
                # chunk streams right after slice 2wt+2
                if k in (2, 4, 6, 8, 10):
                    o0, o1 = RSB_WT_RANGE[(k - 2) // 2]
                    nc.sync.dma_start(out=rsb[:, o0:o1],
                                      in_=rsb_d[:, o0:o1])

            pb = psB.tile([128, NTILE_B * BC], f32)
            nc.vector.memset(pb, 0.0)   # all A/B matmuls accumulate onto 0
            pc = [psC.tile([BC, NH], f32, tag="pc", name=f"pc{h}")
                  for h in range(2)]

            # deterministic/x0 part: out_det = x0aug.T @ RXaug, fp32, as
            # the start=True opener of each pc accumulation group.  Eager:
            # x0t/rxt are the first (tiny) scalar-ring transfers, and these
            # matmuls warm the PE clock gate before the piece stream.
            for h in range(2):
                nc.tensor.matmul(pc[h], x0t, rxt[:, NH * h:NH * (h + 1)],
                                 start=True, stop=False,
                                 skip_group_check=True)

            def emit_stage(wt, h):
                # stage the DVE copy as columns finalize; all 3 output DMAs
                # are issued at the tail (program-order on their engines) so
                # their bytes never steal mid-stream bandwidth, and the 8
                # completion lanes can't stall stream kickoffs behind them
                a, b = CSTAGES[(wt, h)]
                nc.vector.tensor_copy(outsb[:, NH * h + a:NH * h + b],
                                      pc[h][:, a:b])
                if (wt, h) == (NTILE_B - 1, 1):
                    # h1 split across both HWDGE rings so the completion
                    # receipts overlap; h0 (ready since ~60% mark) rides
                    # along on the idle scalar ring
                    nc.scalar.dma_start(out=out_d[:, NH:NH + NH // 2],
                                        in_=outsb[:, NH:NH + NH // 2],
                                        single_packet=True)
                    nc.sync.dma_start(out=out_d[:, NH + NH // 2:NOUT],
                                      in_=outsb[:, NH + NH // 2:NOUT],
                                      single_packet=True)
                    nc.scalar.dma_start(out=out_d[:, 0:NH],
                                        in_=outsb[:, 0:NH],
                                        single_packet=True)

            nstop = {wt: 0 for wt in range(NTILE_B)}
            for p in PIECES:
                k = p['slice']
                eb = SLICE_COLS[k] + BC * (p['s'] - SLICE_OFF[k])
                lhsT = seg_t[k][:, p['off']:p['off'] + p['cols']]
                rhs = seg_t[k][:, eb:eb + BC]
                out = pb[32 * p['m']:32 * p['m'] + p['cols'],
                         BC * p['wt']:BC * (p['wt'] + 1)]
                nc.tensor.matmul(out, lhsT, rhs,
                                 start=p['start'], stop=p['stop'],
                                 tile_position=(0, 32 * p['m']),
                                 skip_group_check=True)
                if not p['stop']:
                    continue
                wt, m = p['wt'], p['m']
                nstop[wt] += 1
                if nstop[wt] == 4:
                    nc.vector.tensor_copy(u50sb[:, BC * wt:BC * (wt + 1)],
                                          pb[:, BC * wt:BC * (wt + 1)])
                    for (h, rel0, keep, off, cst, csp) in CBLOCKS_BY_WT[wt]:
                        nc.tensor.matmul(
                            pc[h][:, rel0:rel0 + keep],
                            u50sb[:, BC * wt:BC * (wt + 1)],
                            rsb[:, off:off + keep],
                            start=cst, stop=csp, skip_group_check=True)
                        if (wt, h) in CSTAGES:
                            emit_stage(wt, h)

    nc.finalize()
    return nc


# ------------------------------------------------------------------ kernel
def kernel(theta, x0, noise, obs_every):
    global _program_cache, _last_results
    from concourse.bass_utils import run_bass_kernel_spmd

    assert int(obs_every) == OBS_EVERY
    theta = np.asarray(theta, np.float32)
    x0 = np.asarray(x0, np.float32)
    noise = np.asarray(noise, np.float32)

    ops = _precompute(theta.astype(np.float64))
    RXaug = ops["RXaug"]                                   # [4, NOUT] float64

    if _program_cache is None:
        _program_cache = _build_program()
    nc = _program_cache

    g2img = ops["g2"]                                      # [128, CTOT] bf16
    nslice = len(SLICE_SUPS)
    soff_cols = np.cumsum([0] + SLICE_COLS)
    seg_cols = [SLICE_COLS[k] + BC * SLICE_SUPS[k] for k in range(nslice)]
    seg_off = np.cumsum([0] + seg_cols)

    rxaug_f = RXaug.astype(np.float32)
    in_maps = []
    for q in range(NCORE):
        sl = slice(BC * q, BC * (q + 1))
        x0aug = np.concatenate([np.ascontiguousarray(x0[sl].T),
                                np.ones((1, BC), np.float32)],
                               axis=0).astype(np.float32)   # [4, BC]
        eps = _pack_eps(noise[sl])                          # [128, 16000] bf16
        stream = np.empty((128, int(seg_off[-1])), ml_dtypes.bfloat16)
        for k in range(nslice):
            a = int(seg_off[k])
            b = a + SLICE_COLS[k]
            stream[:, a:b] = g2img[:, soff_cols[k]:soff_cols[k + 1]]
            stream[:, b:int(seg_off[k + 1])] = \
                eps[:, BC * SLICE_OFF[k]:BC * SLICE_OFF[k + 1]]
        in_maps.append({
            "stream": stream,
            "rsb": ops["rsb"],
            "x0aug": x0aug,
            "rxaug": rxaug_f,
        })

    import os
    trace = bool(os.environ.get("KERNEL_TRACE"))
    res = run_bass_kernel_spmd(nc, in_maps, core_ids=list(range(NCORE)),
                               trace=trace)
    _last_results = res
    out = np.concatenate(
        [res.results[q]["out"].reshape(BC, NOBS, 4) for q in range(NCORE)],
        axis=0)
    return out.astype(np.float32)


# revision 62
# speedup vs baseline: 1.1411x; 1.1411x over previous
"""Trainium2 Bass kernel for the SCON linear-SDE particle scan (v2).

Reference computation: x_{t+1} = (I + DT*W_{t+1}) x_t + DT*b_{t+1} + ds*eps_t
over 10000 steps for B=512 particles with a 3-dim state, observed every 50
steps through a [4,3] projection -> loc_y [512, 201, 4].

The transition matrices depend only on theta (14 scalars), so the whole scan
is a linear map of (x0, eps).  On the host (float64) we precompute propagator
weights; the device reduces the noise tensor in two levels of bf16 PE matmuls
(rel tolerance is 2e-2; bf16 keeps the L2 rel err at ~3e-3):

  level AB: chunk weights G2[c] = S50[w(c),g(c)] @ S10[c] * ds fold the
            10-step chunk propagator AND the within-window chunk propagator,
            so 260 matmuls over the noise produce u50 (per-window noise
            state) directly in PSUM accumulation groups.
  level C:  triangular window->observation propagation + obs projection,
            interleaved per window-tile as u50 tiles complete.

The deterministic/x0 affine part runs as two fp32 K=4 matmuls (x0aug.T @
RXaug) that open the level-C PSUM accumulation groups.  Output halves are
copied out and stored in stages as their contributing window-tiles finish.

B is sharded 64 particles per core across 8 cores (pure data parallel).
Per-core device traffic: ~4.1 MB noise (bf16) + ~1.9 MB weights.  All big
transfers stream on ONE sync-HWDGE FIFO in compute order (g2/eps slices
with per-tile level-C weight chunks inlined before their consumers); the
scalar ring carries only tiny x0 operands and the staged output stores, so
recycling of the 8 DMA completion lanes can never stall a stream kickoff.
"""

import numpy as np
import ml_dtypes

# ---------------------------------------------------------------- constants
T_TOT = 1000.0
DT = 0.1
N = 10001
TEMP_REF = 283.0
TEMP_RISE = 5.0
GAS_R = 0.008314
NSTEP = N - 1            # 10000
B = 512
NCORE = 8
BC = B // NCORE          # 64 particles per core

L1 = 10                  # chunk length (steps)
NC1 = NSTEP // L1        # 1000 chunks
CPW = 5                  # chunks per window
NW = NC1 // CPW          # 200 windows
NOBS = NW + 1            # 201 observations
OBS_EVERY = 50
SUPER = 4                # chunks per matmul (4 x 32 eps rows)
NSUP = NC1 // SUPER      # 250 supergroups
NTILE_B = 5              # u50 window-tiles (40 windows each)
WPB = 10                 # windows per 32-partition block

NOUT = 4 * NOBS          # 804
NH = NOUT // 2           # 402

# stream DMA slices (sup counts), all on the sync HWDGE ring in compute
# order.  Big first slices keep the SDMA engines from starving between
# ~0.65us descriptor-generation kickoffs (the PE start time is slack);
# a small last slice keeps the post-stream tail short.
SLICE_SUPS = [6, 10] + [25] * 8 + [20, 14]
SLICE_OFF = np.cumsum([0] + SLICE_SUPS).tolist()
NSLICE = len(SLICE_SUPS)

_program_cache = None
_last_results = None     # BassKernelResults of the most recent run (for test.py)


# ----------------------------------------------------- static piece metadata
def _sup_pieces():
    """Merged A+B matmul pieces.

    Sup s covers chunks 4s..4s+3 -> windows (4s)//5..(4s+3)//5, all within
    window-tile wt (psum cols 64*wt).  Window w maps to psum partition
    32*((w-40wt)//10) + 3*((w-40wt)%10) + i.  A sup whose windows straddle a
    32-partition block is split into two pieces.  Matmul out partitions must
    start 32-aligned, so each piece's lhsT spans block-row 0..cols (leading
    zero weight columns).  All pieces run start=False against a zeroed PSUM
    bank (accumulate-onto-0 and overwrite are equivalent there).
    """
    pieces = []
    for s in range(NSUP):
        ws = [(4 * s + g) // 5 for g in range(SUPER)]
        wt = ws[0] // 40
        by_m = {}
        for g, w in zip(range(SUPER), ws):
            m = (w - 40 * wt) // WPB
            by_m.setdefault(m, []).append(g)
        for m in sorted(by_m):
            gs = by_m[m]
            whi = ws[gs[-1]]
            rend = 3 * ((whi - 40 * wt) % WPB) + 3
            pieces.append(dict(s=s, wt=wt, m=m, gs=gs, rend=rend))
    for p in pieces:
        # pb is memset to zero, so every matmul can run start=False: rows
        # first touched by a matmul either overwrite or accumulate onto 0.
        p['start'] = False
        p['cols'] = p['rend']
        p['slice'] = int(np.searchsorted(SLICE_OFF, p['s'], side='right')) - 1
    last = {}
    for i, p in enumerate(pieces):
        last[(p['wt'], p['m'])] = i
    for i, p in enumerate(pieces):
        p['stop'] = last[(p['wt'], p['m'])] == i
    cur = {}
    for p in pieces:
        k = p['slice']
        p['off'] = cur.get(k, 0)
        cur[k] = p['off'] + p['cols']
    slice_cols = [cur.get(k, 0) for k in range(len(SLICE_SUPS))]
    return pieces, slice_cols


PIECES, SLICE_COLS = _sup_pieces()
CTOT = sum(SLICE_COLS)


def _rsb_blocks():
    """Nonzero column ranges of each level-C (wt, half) block.

    Window-tile wt covers windows [40wt, 40wt+40); its rows only affect
    observations n >= 40wt+1, i.e. global cols >= 4*(40wt+1).  Returns
    (wt, h, rel0, keep, packed_col_offset)."""
    blocks = []
    off = 0
    for wt in range(NTILE_B):          # wt-major: per-tile contiguous cols
        for h in range(2):
            rel0 = max(0, 4 * (40 * wt + 1) - NH * h)
            if rel0 >= NH:
                continue
            keep = NH - rel0
            blocks.append((wt, h, rel0, keep, off))
            off += keep
    return blocks


RSB_BLOCKS = _rsb_blocks()
NRSB = sum(b[3] for b in RSB_BLOCKS)
RSB_WT_RANGE = {}
for wt, h, rel0, keep, off in RSB_BLOCKS:
    o0, o1 = RSB_WT_RANGE.get(wt, (off, off))
    RSB_WT_RANGE[wt] = (min(o0, off), max(o1, off + keep))
# issue plan per window-tile: (h, rel0, keep, off, start, stop)
CBLOCKS_BY_WT = {wt: [] for wt in range(NTILE_B)}
_last_wt_h = {}
for wt, h, rel0, keep, off in RSB_BLOCKS:
    _last_wt_h[h] = max(_last_wt_h.get(h, 0), wt)
for wt, h, rel0, keep, off in RSB_BLOCKS:
    # start=False always: the fp32 x0-part matmul opens each pc group
    CBLOCKS_BY_WT[wt].append((h, rel0, keep, off, False, wt == _last_wt_h[h]))
for wt in CBLOCKS_BY_WT:
    CBLOCKS_BY_WT[wt].sort()

# output staging: after C(wt,h), pc[h] cols [0, rel0(next tile)) are final.
# CSTAGES[(wt, h)] = (a, b): add det+pc on [a, b) and DMA out cols
# [NH*h + a, NH*h + b) right then, so the kernel tail only ships the last
# stage (160 cols) instead of a whole half.
CSTAGES = {}
for h in range(2):
    tiles = sorted([(wt, rel0) for wt, hh, rel0, _, _ in RSB_BLOCKS
                    if hh == h])
    cur = 0
    for i, (wt, rel0) in enumerate(tiles):
        nxt = tiles[i + 1][1] if i + 1 < len(tiles) else NH
        if nxt > cur:
            CSTAGES[(wt, h)] = (cur, nxt)
            cur = nxt


# ------------------------------------------------------------- host math
def _forcings():
    times = np.linspace(0.0, T_TOT, N)
    temp = (TEMP_REF + TEMP_RISE * times / (80 * 24 * 365)
            + 10 * np.sin(2 * np.pi / 24 * times)
            + 10 * np.sin(2 * np.pi / (24 * 365) * times))
    I_S = 0.001 + 0.0005 * np.sin(2 * np.pi / (24 * 365) * times)
    I_D = 0.0001 + 5e-05 * np.sin(2 * np.pi / (24 * 365) * times)
    return temp, I_S, I_D


def _precompute(theta):
    """float64 propagator weights, packed into the device operand layouts."""
    theta = np.asarray(theta, np.float64)
    (kSr, kDr, kMr, EaS, EaD, EaM, aSD, aDS, aM, aMSC, uM, cS, cD, cM) = theta
    temp, I_S, I_D = _forcings()
    arr = lambda p, Ea: p * np.exp(-Ea / GAS_R * (1.0 / temp - 1.0 / TEMP_REF))
    k_S, k_D, k_M = arr(kSr, EaS), arr(kDr, EaD), arr(kMr, EaM)

    zeros = np.zeros(N)
    A0 = np.stack([-k_S, aDS * k_D, aM * aMSC * k_M])
    A1 = np.stack([aSD * k_S, -(uM + k_D), aM * (1 - aMSC) * k_M])
    A2 = np.stack([zeros, np.full(N, uM), -k_M])
    W = np.stack([A0, A1, A2]).transpose(2, 0, 1)          # [N,3,3]
    bias = np.stack([I_S, I_D, zeros], axis=1)             # [N,3]

    beta = np.clip(np.array([cS, cD, cM]), 1e-6, None)
    ds = np.sqrt(beta * DT)

    M = np.eye(3)[None] + DT * W[1:]                       # [10000,3,3]
    c = DT * bias[1:]                                      # [10000,3]

    # within-chunk suffix products S10[c,tau] = M_end ... M_{tau+1}
    Mc = M.reshape(NC1, L1, 3, 3)
    S10 = np.empty((NC1, L1, 3, 3))
    acc = np.broadcast_to(np.eye(3), (NC1, 3, 3)).copy()
    S10[:, L1 - 1] = acc
    for tau in range(L1 - 2, -1, -1):
        acc = acc @ Mc[:, tau + 1]
        S10[:, tau] = acc
    A10 = S10[:, 0] @ Mc[:, 0]
    b10 = np.einsum('ctij,ctj->ci', S10, c.reshape(NC1, L1, 3))

    # within-window suffix products over chunks
    A10w = A10.reshape(NW, CPW, 3, 3)
    S50 = np.empty((NW, CPW, 3, 3))
    acc = np.broadcast_to(np.eye(3), (NW, 3, 3)).copy()
    S50[:, CPW - 1] = acc
    for g in range(CPW - 2, -1, -1):
        acc = acc @ A10w[:, g + 1]
        S50[:, g] = acc
    A50 = S50[:, 0] @ A10w[:, 0]
    b50 = np.einsum('wgij,wgj->wi', S50, b10.reshape(NW, CPW, 3))

    # deterministic trajectory at obs points (exact, float64)
    detx = np.zeros((NOBS, 3))
    xd = np.zeros(3)
    for w in range(NW):
        xd = A50[w] @ xd + b50[w]
        detx[w + 1] = xd

    # merged chunk->u50 weights: G2[c] = S50[w(c),g(c)] @ S10[c] * ds_j
    G2 = np.einsum('cij,ctjk->ctik', S50.reshape(NC1, 3, 3), S10) * ds
    G2mat = G2.transpose(0, 1, 3, 2).reshape(NC1, 30, 3)   # row 3tau+j, col i

    # observation weights
    sub = np.arange(NOBS) * OBS_EVERY
    C1 = np.stack([(1 - aSD) * k_S[sub], (1 - aDS) * k_D[sub],
                   (1 - aM) * k_M[sub]], axis=1)
    Wobs = np.concatenate([np.broadcast_to(np.eye(3), (NOBS, 3, 3)),
                           C1[:, None, :]], axis=1)        # [NOBS,4,3]

    # Rmat[(w,j),(n,o)] = (Wobs[n] @ A50[n-1] ... A50[w+1]).T  for w < n
    Rmat = np.zeros((3 * NW, NOUT))
    base = np.einsum('noi,ni->no', Wobs, detx).reshape(-1)
    acc = Wobs.copy()
    for w in range(NW - 1, -1, -1):
        Rmat[3 * w:3 * w + 3, 4 * (w + 1):] = \
            acc[w + 1:].transpose(2, 0, 1).reshape(3, -1)
        acc[w + 1:] = acc[w + 1:] @ A50[w]
    RX = acc.transpose(2, 0, 1).reshape(3, -1)             # [3, NOUT]
    RXaug = np.concatenate([RX, base[None]], axis=0)       # [4, NOUT] float64

    # ---------------- pack into device layouts ----------------
    bf = ml_dtypes.bfloat16
    g2img = np.zeros((128, CTOT), np.float32)   # chunk g at rows 32g..32g+30
    soff_cols = np.cumsum([0] + SLICE_COLS)
    G2f = np.asarray(G2mat, np.float32)
    for p in PIECES:
        off = soff_cols[p['slice']] + p['off']
        for g in p['gs']:
            ci = 4 * p['s'] + g
            col0 = off + 3 * ((ci // 5 - 40 * p['wt']) % WPB)
            g2img[32 * g:32 * g + 30, col0:col0 + 3] = G2f[ci]

    # u50 row map: window w, comp j -> row 32*((w%40)//10) + 3*(w%10) + j,
    #              col 64*(w//40) + b
    rsb = np.zeros((128, NRSB), np.float32)
    for wt, h, rel0, keep, off in RSB_BLOCKS:
        for rho in range(128):
            q = rho % 32
            if q >= 30:
                continue
            w = WPB * (4 * wt + rho // 32) + q // 3
            j = q % 3
            rsb[rho, off:off + keep] = \
                Rmat[3 * w + j, NH * h + rel0:NH * h + rel0 + keep]

    return dict(g2=g2img.astype(bf), rsb=rsb.astype(bf), RXaug=RXaug)


def _pack_eps(noise_core):
    """[64,10000,3] f32 -> [128, 250*64] bf16: row 32g + (3tau+j),
    col 64s + b = eps[b, t, j] for t = 40s + 10g + tau; rows 32g+30/31 pad."""
    a = noise_core.reshape(BC, NSTEP * 3).T          # [30000, 64] view
    a = np.ascontiguousarray(a).reshape(NSUP, SUPER, 30, BC)
    out = np.zeros((SUPER, 32, NSUP, BC), ml_dtypes.bfloat16)
    out[:, :30] = a.transpose(1, 2, 0, 3).astype(ml_dtypes.bfloat16)
    return out.reshape(128, NSUP * BC)


# ------------------------------------------------------------ bass program
def _build_program(**bass_kwargs):
    import concourse.bass as bass
    import concourse.tile as tile
    from concourse import bacc, mybir

    f32 = mybir.dt.float32
    bf16 = mybir.dt.bfloat16
    nc = bacc.Bacc(None, target_bir_lowering=False, **bass_kwargs)

    # per-slice stream segment: [g2_k cols | eps_k cols], one DMA each.
    # 128 rows with chunk g at rows 32g..32g+30 (rows 32g+30/31 zero pad):
    # 120-partition DMAs measure ~230 GB/s vs ~400 GB/s for 128-partition,
    # so shipping the pad rows is the faster option.
    seg_cols = [SLICE_COLS[k] + BC * SLICE_SUPS[k]
                for k in range(len(SLICE_SUPS))]
    seg_off = np.cumsum([0] + seg_cols)

    stream_d = nc.dram_tensor("stream", [128, int(seg_off[-1])], bf16,
                              kind="ExternalInput")
    rsb_d = nc.dram_tensor("rsb", [128, NRSB], bf16, kind="ExternalInput")
    x0_d = nc.dram_tensor("x0aug", [4, BC], f32, kind="ExternalInput")
    rx_d = nc.dram_tensor("rxaug", [4, NOUT], f32, kind="ExternalInput")
    out_d = nc.dram_tensor("out", [BC, NOUT], f32, kind="ExternalOutput")

    with tile.TileContext(nc) as tc:
        with (
            tc.tile_pool(name="consts", bufs=1) as consts,
            tc.tile_pool(name="epsp", bufs=1) as epsp,
            tc.tile_pool(name="psB", bufs=1, space="PSUM") as psB,
            tc.tile_pool(name="psC", bufs=2, space="PSUM") as psC,
        ):
            rsb = consts.tile([128, NRSB], bf16)
            x0t = consts.tile([4, BC], f32)
            rxt = consts.tile([4, NOUT], f32)
            u50sb = consts.tile([128, NTILE_B * BC], bf16)
            outsb = consts.tile([BC, NOUT], f32)
            seg_t = [epsp.tile([128, sc], bf16, tag=f"seg{k}",
                               name=f"seg{k}")
                     for k, sc in enumerate(seg_cols)]

            # scalar (qAct) HWDGE ring: only the tiny x0 operands (plus the
            # staged output stores later).  Keeping big transfers off this
            # ring stops sync-ring kickoffs from serializing behind them
            # when the 8 DMA completion lanes are recycled.
            nc.scalar.dma_start(out=x0t, in_=x0_d[:])
            nc.scalar.dma_start(out=rxt, in_=rx_d[:])
            # sync (qSP) HWDGE ring: one DMA per slice (weights + noise) in
            # compute order; level-C weight chunks for tile wt are streamed
            # inline just before the slices that complete the tile, keeping
            # all big transfers on one FIFO (in-order completions mean the
            # 8 recycled completion lanes can never stall a later kickoff)
            for k in range(NSLICE):
                nc.sync.dma_start(
                    out=seg_t[k],
                    in_=stream_d[:, int(seg_off[k]):int(seg_off[k + 1])])
                # tile wt completes in slice 2wt+3; its level-C weight
                # chunk streams right after slice 2wt+2
                if k in (2, 4, 6, 8, 10):
                    o0, o1 = RSB_WT_RANGE[(k - 2) // 2]
                    nc.sync.dma_start(out=rsb[:, o0:o1],
                                      in_=rsb_d[:, o0:o1])

            pb = psB.tile([128, NTILE_B * BC], f32)
            nc.vector.memset(pb, 0.0)   # all A/B matmuls accumulate onto 0
            pc = [psC.tile([BC, NH], f32, tag="pc", name=f"pc{h}")
                  for h in range(2)]

            # deterministic/x0 part: out_det = x0aug.T @ RXaug, fp32, as
            # the start=True opener of each pc accumulation group.  Eager:
            # x0t/rxt are the first (tiny) scalar-ring transfers, and these
            # matmuls warm the PE clock gate before the piece stream.
            for h in range(2):
                nc.tensor.matmul(pc[h], x0t, rxt[:, NH * h:NH * (h + 1)],
                                 start=True, stop=False,
                                 skip_group_check=True)

            def emit_stage(wt, h):
                # stage the DVE copy as columns finalize, but only 3 output
                # DMAs total (extra DMAs recycle the 8 completion lanes and
                # can stall later stream kickoffs behind compute)
                a, b = CSTAGES[(wt, h)]
                nc.vector.tensor_copy(outsb[:, NH * h + a:NH * h + b],
                                      pc[h][:, a:b])
                if (wt, h) == (2, 0):
                    nc.scalar.dma_start(out=out_d[:, 0:NH],
                                        in_=outsb[:, 0:NH],
                                        single_packet=True)
                elif (wt, h) == (NTILE_B - 1, 1):
                    # final half: split across both HWDGE rings so the two
                    # completion receipts overlap on the kernel tail
                    nc.scalar.dma_start(out=out_d[:, NH:NH + NH // 2],
                                        in_=outsb[:, NH:NH + NH // 2],
                                        single_packet=True)
                    nc.sync.dma_start(out=out_d[:, NH + NH // 2:NOUT],
                                      in_=outsb[:, NH + NH // 2:NOUT],
                                      single_packet=True)

            nstop = {wt: 0 for wt in range(NTILE_B)}
            for p in PIECES:
                k = p['slice']
                eb = SLICE_COLS[k] + BC * (p['s'] - SLICE_OFF[k])
                lhsT = seg_t[k][:, p['off']:p['off'] + p['cols']]
                rhs = seg_t[k][:, eb:eb + BC]
                out = pb[32 * p['m']:32 * p['m'] + p['cols'],
                         BC * p['wt']:BC * (p['wt'] + 1)]
                nc.tensor.matmul(out, lhsT, rhs,
                                 start=p['start'], stop=p['stop'],
                                 tile_position=(0, 32 * p['m']),
                                 skip_group_check=True)
                if not p['stop']:
                    continue
                wt, m = p['wt'], p['m']
                nstop[wt] += 1
                if nstop[wt] == 4:
                    nc.vector.tensor_copy(u50sb[:, BC * wt:BC * (wt + 1)],
                                          pb[:, BC * wt:BC * (wt + 1)])
                    for (h, rel0, keep, off, cst, csp) in CBLOCKS_BY_WT[wt]:
                        nc.tensor.matmul(
                            pc[h][:, rel0:rel0 + keep],
                            u50sb[:, BC * wt:BC * (wt + 1)],
                            rsb[:, off:off + keep],
                            start=cst, stop=csp, skip_group_check=True)
                        if (wt, h) in CSTAGES:
                            emit_stage(wt, h)

    nc.finalize()
    return nc


# ------------------------------------------------------------------ kernel
def kernel(theta, x0, noise, obs_every):
    global _program_cache, _last_results
    from concourse.bass_utils import run_bass_kernel_spmd

    assert int(obs_every) == OBS_EVERY
    theta = np.asarray(theta, np.float32)
    x0 = np.asarray(x0, np.float32)
    noise = np.asarray(noise, np.float32)

    ops = _precompute(theta.astype(np.float64))
    RXaug = ops["RXaug"]                                   # [4, NOUT] float64

    if _program_cache is None:
        _program_cache = _build_program()
    nc = _program_cache

    g2img = ops["g2"]                                      # [128, CTOT] bf16
    nslice = len(SLICE_SUPS)
    soff_cols = np.cumsum([0] + SLICE_COLS)
    seg_cols = [SLICE_COLS[k] + BC * SLICE_SUPS[k] for k in range(nslice)]
    seg_off = np.cumsum([0] + seg_cols)

    rxaug_f = RXaug.astype(np.float32)
    in_maps = []
    for q in range(NCORE):
        sl = slice(BC * q, BC * (q + 1))
        x0aug = np.concatenate([np.ascontiguousarray(x0[sl].T),
                                np.ones((1, BC), np.float32)],
                               axis=0).astype(np.float32)   # [4, BC]
        eps = _pack_eps(noise[sl])                          # [128, 16000] bf16
        stream = np.empty((128, int(seg_off[-1])), ml_dtypes.bfloat16)
        for k in range(nslice):
            a = int(seg_off[k])
            b = a + SLICE_COLS[k]
            stream[:, a:b] = g2img[:, soff_cols[k]:soff_cols[k + 1]]
            stream[:, b:int(seg_off[k + 1])] = \
                eps[:, BC * SLICE_OFF[k]:BC * SLICE_OFF[k + 1]]
        in_maps.append({
            "stream": stream,
            "rsb": ops["rsb"],
            "x0aug": x0aug,
            "rxaug": rxaug_f,
        })

    import os
    trace = bool(os.environ.get("KERNEL_TRACE"))
    res = run_bass_kernel_spmd(nc, in_maps, core_ids=list(range(NCORE)),
                               trace=trace)
    _last_results = res
    out = np.concatenate(
        [res.results[q]["out"].reshape(BC, NOBS, 4) for q in range(NCORE)],
        axis=0)
    return out.astype(np.float32)
